# revision 7
# baseline (speedup 1.0000x reference)
# Multi-head attention (B=2, N=2048, C=1024, H=16) on 8 trn2 NeuronCores.
#
# Sharding: core = (batch b = core//4, head-group hg = core%4, 4 heads each).
# Each core computes qkv/attention/proj for its 4 heads of its batch and
# returns a partial projection output [N, C]; the host sums the 4 partials
# per batch and adds b_proj.
#
# Per-core device pipeline (all matmuls in float32r, full-rate at N>=256):
#   1. x [N,C] -> PE-transpose -> xT [C,N]                 (fp32 transposes)
#   2. qkvT[768, N] = Wsel @ x^T   (lhsT=wqkvT, rhs=xT)    -> q^T,k^T,v^T
#   3. v' = [v | 1] natural layout via PE-transpose of v^T
#   4. per (head) unit: S^T[j,i] = k @ q^T ; E=exp(S*scale) on ACT;
#      O'^T[65, N] += v'^T @ E^T  (row 64 = softmax denominator)
#      normalize with 1/rowsum broadcast (gpsimd partition_broadcast)
#   5. proj partial: out[i,e] = sum_ch O^T[ch,i] * wprojT[ch,e]
import sys

import numpy as np

if "/opt/trn_rl_repo" not in sys.path:
    sys.path.insert(0, "/opt/trn_rl_repo")

B, NSEQ, C = 2, 2048, 1024
H, HD = 16, 64
P = 128
SCALE = HD**-0.5

_cache = {}


def _build(nseq):
    from contextlib import ExitStack

    import concourse.tile as tile
    from concourse import bacc, mybir
    from concourse.masks import make_identity

    f32 = mybir.dt.float32
    f32r = mybir.dt.float32r
    EXP = mybir.ActivationFunctionType.Exp

    NIT = nseq // P          # i tiles (output rows / queries)
    NJT = nseq // P          # j tiles (keys)
    QCH = min(512, nseq)     # matmul moving-dim chunk
    SW = min(1024, nseq)     # S^T psum tile width (2 banks)
    NSW = nseq // SW
    NOB = nseq // QCH        # number of O' psum tiles
    ECH = 512                # proj output chunk

    nc = bacc.Bacc("TRN2", target_bir_lowering=False, debug=False, num_devices=8)
    x_d = nc.dram_tensor("x", [nseq, C], f32, kind="ExternalInput")
    wq_d = nc.dram_tensor("wqkvT", [C, 6 * P], f32r, kind="ExternalInput")
    wp_d = nc.dram_tensor("wprojT", [P, 2, C], f32r, kind="ExternalInput")
    out_d = nc.dram_tensor("out", [nseq, C], f32, kind="ExternalOutput")

    cp_state = [0]

    def cp(out, in_):
        # alternate PSUM->SBUF copies between DVE and ACT
        cp_state[0] ^= 1
        if cp_state[0]:
            nc.vector.tensor_copy(out, in_)
        else:
            nc.scalar.copy(out, in_)

    with tile.TileContext(nc) as tc, ExitStack() as ctx:
        persist = ctx.enter_context(tc.tile_pool(name="persist", bufs=1))

        wp_sb = persist.tile([P, 2, C], f32r)
        nc.sync.dma_start(wp_sb, wp_d.ap())
        qkT = persist.tile([P, 4, nseq], f32r)
        ones_f32 = persist.tile([P, 1], f32)
        nc.vector.memset(ones_f32, 1.0)
        v1 = persist.tile([P, 4, NJT, HD + 1], f32r)

        # ======== scope A: phases 1-3 ========
        with (
            tc.tile_pool(name="scopeA", bufs=1) as scopeA,
            tc.tile_pool(name="xin", bufs=2) as xin,
            tc.tile_pool(name="psA", bufs=2, space="PSUM") as psA,
            tc.tile_pool(name="psAv", bufs=2, space="PSUM") as psAv,
            tc.tile_pool(name="psQ", bufs=2, space="PSUM") as psQ,
        ):
            ident = scopeA.tile([P, P], f32)
            make_identity(nc, ident)
            wq_sb = scopeA.tile([P, 8, 6 * P], f32r)
            nc.sync.dma_start(wq_sb, wq_d.ap().rearrange("(co p) d -> p co d", p=P))
            xT = scopeA.tile([P, 8, nseq], f32r)
            vT = scopeA.tile([P, 2, nseq], f32)

            # ---- Phase 1: transpose x into xT[c_part, c_outer, i] ----
            for it in range(NIT):
                xt = xin.tile([P, C], f32)
                nc.sync.dma_start(xt, x_d[it * P : (it + 1) * P, :])
                for cg in range(2):
                    ps = psA.tile([P, 4, P], f32, tag="psA")
                    for k in range(4):
                        cch = cg * 4 + k
                        nc.tensor.transpose(
                            ps[:, k, :], xt[:, cch * P : (cch + 1) * P], ident
                        )
                    cp(xT[:, cg * 4 : cg * 4 + 4, it * P : (it + 1) * P], ps)

            # ---- Phase 2: qkvT[p, mt, i] ----
            for mt in range(6):
                for nch in range(nseq // QCH):
                    ps = psQ.tile([P, QCH], f32, tag="psQ")
                    for co in range(8):
                        nc.tensor.matmul(
                            ps,
                            lhsT=wq_sb[:, co, mt * P : (mt + 1) * P],
                            rhs=xT[:, co, nch * QCH : (nch + 1) * QCH],
                            start=(co == 0),
                            stop=(co == 7),
                        )
                    dest = (
                        qkT[:, mt, nch * QCH : (nch + 1) * QCH]
                        if mt < 4
                        else vT[:, mt - 4, nch * QCH : (nch + 1) * QCH]
                    )
                    cp(dest, ps)

            # ---- Phase 3: v' natural [j_part, u, jt, 65] with ones column ----
            nc.vector.tensor_copy(
                v1[:, :, :, HD : HD + 1],
                ones_f32[:, None, None, :].to_broadcast([P, 4, NJT, 1]),
            )
            VB = min(4, NJT)
            for u in range(4):
                pb = 64 * (u % 2)
                vT_u = vT[pb : pb + 64, u // 2, :]
                for jg in range(NJT // VB):
                    ps = psAv.tile([P, VB, HD], f32, tag="psAv")
                    for k in range(VB):
                        jt = jg * VB + k
                        nc.tensor.transpose(
                            ps[:, k, :],
                            vT_u[:, jt * P : (jt + 1) * P],
                            ident[pb : pb + 64, pb : pb + 64],
                        )
                    cp(v1[:, u, jg * VB : jg * VB + VB, 0:HD], ps)

        # ======== scope B/C: attention + proj ========
        with tc.tile_pool(name="otpool", bufs=1) as otpool:
            OT = otpool.tile([P, 2, nseq], f32r)

            with (
                tc.tile_pool(name="epool", bufs=3) as epool,
                tc.tile_pool(name="small", bufs=1) as small,
                tc.tile_pool(name="psS", bufs=2, space="PSUM") as psS,
                tc.tile_pool(name="psO", bufs=4, space="PSUM") as psO,
            ):
                # ---- Phase 4: attention per unit ----
                for u in range(4):
                    pb = 64 * (u % 2)
                    qT_u = qkT[pb : pb + 64, u // 2, :]
                    kT_u = qkT[pb : pb + 64, 2 + u // 2, :]
                    psO_tiles = [
                        psO.tile([P, QCH], f32, tag="psO", name=f"psO_{u}_{q}")
                        for q in range(NOB)
                    ]
                    for jt in range(NJT):
                        for sw in range(NSW):
                            ps = psS.tile([P, SW], f32, tag="psS")
                            for q2 in range(SW // QCH):
                                nc.tensor.matmul(
                                    ps[:, q2 * QCH : (q2 + 1) * QCH],
                                    lhsT=kT_u[:, jt * P : (jt + 1) * P],
                                    rhs=qT_u[
                                        :,
                                        sw * SW + q2 * QCH : sw * SW + (q2 + 1) * QCH,
                                    ],
                                    start=True,
                                    stop=True,
                                )
                            et = epool.tile([P, SW], f32r, tag="epool")
                            nc.scalar.activation(et, ps, EXP, scale=SCALE)
                            for q2 in range(SW // QCH):
                                q = sw * (SW // QCH) + q2
                                nc.tensor.matmul(
                                    psO_tiles[q][0 : HD + 1, :],
                                    lhsT=v1[:, u, jt, :],
                                    rhs=et[:, q2 * QCH : (q2 + 1) * QCH],
                                    start=(jt == 0),
                                    stop=(jt == NJT - 1),
                                )
                    # normalize: OT[ch, i] = O'[ch, i] / rowsum[i]
                    recip = small.tile([1, nseq], f32, tag="recip")
                    for q in range(NOB):
                        nc.vector.reciprocal(
                            recip[:, q * QCH : (q + 1) * QCH],
                            psO_tiles[q][HD : HD + 1, :],
                        )
                    bcast = small.tile([64, nseq], f32, tag="bcast")
                    nc.gpsimd.partition_broadcast(bcast, recip)
                    for q in range(NOB):
                        nc.vector.tensor_mul(
                            OT[pb : pb + 64, u // 2, q * QCH : (q + 1) * QCH],
                            psO_tiles[q][0:64, :],
                            bcast[:, q * QCH : (q + 1) * QCH],
                        )

            with (
                tc.tile_pool(name="opool", bufs=3) as opool,
                tc.tile_pool(name="psP", bufs=2, space="PSUM") as psP,
            ):
                # ---- Phase 5: proj partial out[i, e] ----
                for it in range(NIT):
                    for ech in range(C // ECH):
                        ps = psP.tile([P, ECH], f32, tag="psP")
                        for co in range(2):
                            nc.tensor.matmul(
                                ps,
                                lhsT=OT[:, co, it * P : (it + 1) * P],
                                rhs=wp_sb[:, co, ech * ECH : (ech + 1) * ECH],
                                start=(co == 0),
                                stop=(co == 1),
                            )
                        ot = opool.tile([P, ECH], f32, tag="opool")
                        cp(ot, ps)
                        nc.sync.dma_start(
                            out_d[it * P : (it + 1) * P, ech * ECH : (ech + 1) * ECH],
                            ot,
                        )

    nc.compile()
    return nc


def get_nc(nseq=NSEQ):
    if nseq not in _cache:
        _cache[nseq] = _build(nseq)
    return _cache[nseq]


def make_in_maps(x, w_qkv, w_proj, nseq=NSEQ):
    x = np.ascontiguousarray(x, dtype=np.float32)
    w_qkv = np.ascontiguousarray(w_qkv, dtype=np.float32)
    w_proj = np.ascontiguousarray(w_proj, dtype=np.float32)
    in_maps = []
    for core in range(8):
        b, hg = core // 4, core % 4
        hs = 4 * hg
        wsel = np.empty((6, P, C), np.float32)
        for mt in range(6):
            t, half = mt // 2, mt % 2
            r0 = t * C + (hs + 2 * half) * HD
            wsel[mt] = w_qkv[r0 : r0 + P, :]
        wqkvT = np.ascontiguousarray(wsel.transpose(2, 0, 1).reshape(C, 6 * P))
        wp = np.empty((P, 2, C), np.float32)
        for co in range(2):
            c0 = (hs + 2 * co) * HD
            wp[:, co, :] = w_proj[:, c0 : c0 + P].T
        in_maps.append(
            {"x": np.ascontiguousarray(x[b, :nseq]), "wqkvT": wqkvT, "wprojT": wp}
        )
    return in_maps


def kernel(x, w_qkv, w_proj, b_proj):
    from concourse.bass_utils import run_bass_kernel_spmd

    nc = get_nc()
    in_maps = make_in_maps(x, w_qkv, w_proj)
    res = run_bass_kernel_spmd(nc, in_maps, core_ids=list(range(8)))
    parts = [r["out"] for r in res.results]
    out = np.stack(
        [
            parts[0] + parts[1] + parts[2] + parts[3],
            parts[4] + parts[5] + parts[6] + parts[7],
        ],
        axis=0,
    )
    return (out + np.asarray(b_proj, np.float32)).astype(np.float32)
